# revision 11
# baseline (speedup 1.0000x reference)
"""Multi-head attention (B=2, T=4096, D=512, H=8) on 8 TRN2 NeuronCores.

Sharding: data-parallel over batch (2) x tensor-parallel over head-pairs (4).
Core c handles batch c//4 and heads {2*(c%4), 2*(c%4)+1}. Each core returns a
[T, D] partial of (attn_out @ Wo.T); the host sums the 4 partials per batch and
adds bo. No on-device collectives.

Math per core (d_local = 128 dims = 2 heads x 64):
  QT[d,t] = (Wq_slice @ x_b.T)        via lhsT=Wq_slice.T tiles (host-pretransposed)
  S_T[s,t] = sum_d KT[d,s] QT[d,t]    (2 heads row-packed on the PE, K=64 each)
  E = exp(S_T / 64)                   (ScalarE, scale fused; logits are O(1))
  AV[m,t] = V_aug.T @ E               V_aug = [V_nat | ones] -> row 64 = softmax denom
  avt[d,t] = AV[d,t] * (1/denom[t])   (DMA partition-broadcast of reciprocals)
  partial[t,e] = sum_d avt[d,t] WoT[d,e]
"""

import ml_dtypes
import numpy as np

import concourse.bass as bass
import concourse.bacc as bacc
import concourse.mybir as mybir
import concourse.tile as tile
from concourse.bass import ds, ts
from concourse.masks import make_identity

B, T, D, H = 2, 4096, 512, 8
DH = D // H  # 64
P = 128
NCORES = 8
NT = T // 512   # 8 t-chunks of 512
ST = T // P     # 32 s-tiles of 128
KC = D // P     # 4 contraction tiles for projections

F32 = mybir.dt.float32
F32R = mybir.dt.float32r
BF16 = mybir.dt.bfloat16
AX = mybir.AluOpType


def build_bass(sgroup=2, ebufs=6, sbufs=3, loop_n=1, runtime_loop=False):
    """Build the single-core Bass program (same program runs on all 8 cores)."""
    nc = bacc.Bacc("TRN2", target_bir_lowering=False, debug=False,
                   num_devices=NCORES)

    xT_d = nc.dram_tensor("xT", [KC, P, T], BF16, kind="ExternalInput").ap()
    wqT_d = nc.dram_tensor("wqT", [KC, P, P], BF16, kind="ExternalInput").ap()
    wkT_d = nc.dram_tensor("wkT", [KC, P, P], BF16, kind="ExternalInput").ap()
    wvT_d = nc.dram_tensor("wvT", [KC, P, P], BF16, kind="ExternalInput").ap()
    wo0_d = nc.dram_tensor("woT0", [DH, D], BF16, kind="ExternalInput").ap()
    wo1_d = nc.dram_tensor("woT1", [DH, D], BF16, kind="ExternalInput").ap()
    outp_d = nc.dram_tensor("outp", [T, D], F32, kind="ExternalOutput").ap()
    nit_d = None
    if runtime_loop:
        nit_d = nc.dram_tensor("niter", [1, 1], mybir.dt.uint32,
                               kind="ExternalInput").ap()

    with tile.TileContext(nc) as tc:
        if runtime_loop:
            regs = nc.alloc_registers("nit", bass.OrderedSet(list(nc.engines.keys())))
            for reg in regs.handles:
                nc.reg_load(reg, nit_d[0:1, 0:1])
            bound = nc.snap(regs, min_val=0, max_val=100000)
            with tc.For_i(0, bound, 1):
                _program(tc, xT_d, wqT_d, wkT_d, wvT_d, wo0_d, wo1_d, outp_d,
                         sgroup=sgroup, ebufs=ebufs, sbufs=sbufs)
        elif loop_n > 1:
            with tc.For_i(0, loop_n, 1):
                _program(tc, xT_d, wqT_d, wkT_d, wvT_d, wo0_d, wo1_d, outp_d,
                         sgroup=sgroup, ebufs=ebufs, sbufs=sbufs)
        else:
            _program(tc, xT_d, wqT_d, wkT_d, wvT_d, wo0_d, wo1_d, outp_d,
                     sgroup=sgroup, ebufs=ebufs, sbufs=sbufs)
    nc.compile()
    return nc


def _program(tc, xT_d, wqT_d, wkT_d, wvT_d, wo0_d, wo1_d, outp_d,
             sgroup, ebufs, sbufs):
    nc = tc.nc
    from contextlib import ExitStack
    with ExitStack() as ctx:
        const = ctx.enter_context(tc.tile_pool(name="const", bufs=1))
        big = ctx.enter_context(tc.tile_pool(name="big", bufs=1))
        epool = ctx.enter_context(tc.tile_pool(name="exp", bufs=ebufs))
        rpool = ctx.enter_context(tc.tile_pool(name="recip", bufs=4))
        drpool = ctx.enter_context(tc.tile_pool(name="dramr", bufs=4, space="DRAM"))
        opool = ctx.enter_context(tc.tile_pool(name="outs", bufs=3))
        spool = ctx.enter_context(tc.tile_pool(name="spsum", bufs=sbufs, space="PSUM"))
        avp0 = ctx.enter_context(tc.tile_pool(name="avp0", bufs=1, space="PSUM"))
        avp1 = ctx.enter_context(tc.tile_pool(name="avp1", bufs=1, space="PSUM"))

        SW = 512 * sgroup  # free width of one score psum tile

        # ---- constants / inputs ----
        xt = big.tile([P, KC, T], BF16)       # x_b.T, c on partitions (64KB/part)
        for k in range(KC):
            nc.sync.dma_start(xt[:, k, :], xT_d[k])

        wq = const.tile([P, KC, P], BF16)
        wk = const.tile([P, KC, P], BF16)
        wv = const.tile([P, KC, P], BF16)
        for w_sb, w_d in ((wq, wqT_d), (wk, wkT_d), (wv, wvT_d)):
            for k in range(KC):
                nc.sync.dma_start(w_sb[:, k, :], w_d[k])
        wo0 = const.tile([DH, D], BF16)
        nc.sync.dma_start(wo0[:], wo0_d)
        wo1 = const.tile([DH, D], BF16)
        nc.sync.dma_start(wo1[:], wo1_d)
        ident = const.tile([P, P], BF16)
        make_identity(nc, ident)

        # ---- phase 0: projections (fp32r, [d_local, t] layout) ----
        qt = big.tile([P, T], BF16)
        ktt = big.tile([P, T], BF16)
        vtmp = big.tile([P, T], BF16)
        for w_sb, dst in ((wq, qt), (wk, ktt), (wv, vtmp)):
            for i in range(NT):
                ps = spool.tile([P, 512], F32, tag="sc")
                for k in range(KC):
                    nc.tensor.matmul(
                        ps,
                        w_sb[:, k, :],
                        xt[:, k, ds(i * 512, 512)],
                        start=(k == 0), stop=(k == KC - 1),
                    )
                # evict (+ cast to bf16); biases are zero for this problem
                nc.vector.tensor_copy(dst[:, ds(i * 512, 512)], ps)

        # V -> natural layout [s, d] via PE transpose; append ones column
        vaug0 = big.tile([P, ST, DH + 1], BF16)
        vaug1 = big.tile([P, ST, DH + 1], BF16)
        nc.vector.memset(vaug0[:, :, DH:DH + 1], 1.0)
        nc.vector.memset(vaug1[:, :, DH:DH + 1], 1.0)
        for s in range(ST):
            tp = spool.tile([P, P], BF16, tag="sc")
            nc.tensor.transpose(tp, vtmp[:, ds(s * P, P)], ident)
            nc.vector.tensor_copy(vaug0[:, s, 0:DH], tp[:, 0:DH])
            nc.vector.tensor_copy(vaug1[:, s, 0:DH], tp[:, DH:P])

        # ---- phase 1: attention ----
        avt0 = big.tile([DH, T], BF16)
        avt1 = big.tile([DH, T], BF16)
        NSG = ST // sgroup
        for i in range(NT):  # t-chunk of 512
            tsl = ds(i * 512, 512)
            av0 = avp0.tile([DH + 1, 512], F32, tag="av0")
            av1 = avp1.tile([DH + 1, 512], F32, tag="av1")
            for g in range(NSG):
                sp0 = spool.tile([P, SW], F32, tag="sc")
                sp1 = spool.tile([P, SW], F32, tag="sc")
                for j in range(sgroup):
                    s = g * sgroup + j
                    ssl = ds(s * P, P)
                    nc.tensor.matmul(sp0[:, ds(j * 512, 512)],
                                     ktt[0:DH, ssl], qt[0:DH, tsl],
                                     start=True, stop=True)
                    nc.tensor.matmul(sp1[:, ds(j * 512, 512)],
                                     ktt[DH:P, ssl], qt[DH:P, tsl],
                                     start=True, stop=True)
                e0 = epool.tile([P, SW], BF16, tag="e")
                nc.scalar.activation(e0, sp0, mybir.ActivationFunctionType.Exp,
                                     scale=1.0 / DH)
                e1 = epool.tile([P, SW], BF16, tag="e")
                nc.scalar.activation(e1, sp1, mybir.ActivationFunctionType.Exp,
                                     scale=1.0 / DH)
                for j in range(sgroup):
                    s = g * sgroup + j
                    nc.tensor.matmul(av0, vaug0[:, s, :], e0[:, ds(j * 512, 512)],
                                     start=(s == 0), stop=(s == ST - 1))
                    nc.tensor.matmul(av1, vaug1[:, s, :], e1[:, ds(j * 512, 512)],
                                     start=(s == 0), stop=(s == ST - 1))
            # normalize: avt_h[:, chunk] = av_h[0:64] * (1/denom) (bcast over d)
            for av, avt in ((av0, avt0), (av1, avt1)):
                rt = rpool.tile([DH + 1, 512], F32, tag="rt")
                nc.vector.reciprocal(rt[DH:DH + 1, :], av[DH:DH + 1, :])
                rd = drpool.tile([1, 512], F32, tag="rd")
                nc.sync.dma_start(rd[:], rt[DH:DH + 1, :])
                rb = rpool.tile([DH, 512], F32, tag="rb")
                nc.sync.dma_start(rb[:], rd[:].to_broadcast((DH, 512)))
                nc.vector.tensor_tensor(avt[:, tsl], av[0:DH, :], rb[:], AX.mult)

        # ---- phase 2: output projection (fp32r) ----
        for t in range(ST):
            po = spool.tile([P, D], F32, tag="sc")
            nc.tensor.matmul(po, avt0[:, ds(t * P, P)], wo0[:],
                             start=True, stop=False)
            nc.tensor.matmul(po, avt1[:, ds(t * P, P)], wo1[:],
                             start=False, stop=True)
            ot = opool.tile([P, D], F32, tag="o")
            nc.vector.tensor_copy(ot, po)
            nc.sync.dma_start(outp_d[ds(t * P, P), :], ot)


def make_in_maps(query, Wq, bq, Wk, bk, Wv, bv, Wo):
    """Host-side sharding: per-core input dict."""
    query = np.ascontiguousarray(query, dtype=np.float32)
    in_maps = []
    for c in range(NCORES):
        b, hp = c // 4, c % 4
        sl = slice(hp * P, (hp + 1) * P)
        xT = np.ascontiguousarray(query[b].T).reshape(KC, P, T)
        wqT = np.ascontiguousarray(Wq[sl, :].T).reshape(KC, P, P)
        wkT = np.ascontiguousarray(Wk[sl, :].T).reshape(KC, P, P)
        wvT = np.ascontiguousarray(Wv[sl, :].T).reshape(KC, P, P)
        woT = np.ascontiguousarray(Wo[:, sl].T)  # [128, 512]
        bf = ml_dtypes.bfloat16
        in_maps.append({
            "xT": np.ascontiguousarray(xT.astype(bf)),
            "wqT": np.ascontiguousarray(wqT.astype(bf)),
            "wkT": np.ascontiguousarray(wkT.astype(bf)),
            "wvT": np.ascontiguousarray(wvT.astype(bf)),
            "woT0": np.ascontiguousarray(woT[0:DH, :].astype(bf)),
            "woT1": np.ascontiguousarray(woT[DH:P, :].astype(bf)),
        })
    return in_maps


def gather(results, bo):
    """Sum per-core partials into the full [B, T, D] output."""
    out = np.zeros((B, T, D), dtype=np.float32)
    for c in range(NCORES):
        out[c // 4] += results[c]["outp"]
    out += np.asarray(bo, dtype=np.float32)[None, None, :]
    return out


_NC_CACHE = {}


def _get_nc(**kw):
    key = tuple(sorted(kw.items()))
    if key not in _NC_CACHE:
        _NC_CACHE[key] = build_bass(**kw)
    return _NC_CACHE[key]


def _numpy_fallback(query, mask, Wq, bq, Wk, bk, Wv, bv, Wo, bo):
    """Exact reference on host; only used if inputs violate kernel assumptions
    (nonzero mask entries / masked rows), which the standard problem never
    produces."""
    query = np.asarray(query, dtype=np.float64)
    scale = np.sqrt(DH)
    out = np.zeros((B, T, D))
    for b in range(B):
        x = query[b]
        q = (x @ Wq.T + bq) / scale
        k = x @ Wk.T + bk
        v = x @ Wv.T + bv
        qh = q.reshape(T, H, DH).transpose(1, 0, 2)
        kh = k.reshape(T, H, DH).transpose(1, 0, 2)
        vh = v.reshape(T, H, DH).transpose(1, 0, 2)
        o = np.zeros((H, T, DH))
        for h in range(H):
            s = qh[h] @ kh[h].T
            s = np.where((np.asarray(mask[b]) == 0)[:, None], -1e20, s)
            s = s / scale
            s = s - s.max(axis=-1, keepdims=True)
            e = np.exp(s)
            a = e / e.sum(axis=-1, keepdims=True)
            o[h] = a @ vh[h]
        oo = o.transpose(1, 0, 2).reshape(T, D)
        out[b] = oo @ Wo.T + bo
    return out.astype(np.float32)


def kernel(query, mask, Wq, bq, Wk, bk, Wv, bv, Wo, bo, **_ignored):
    query = np.asarray(query)
    mask = np.asarray(mask)
    Wq, bq = np.asarray(Wq), np.asarray(bq)
    Wk, bk = np.asarray(Wk), np.asarray(bk)
    Wv, bv = np.asarray(Wv), np.asarray(bv)
    Wo, bo = np.asarray(Wo), np.asarray(bo)

    if np.any(mask == 0) or np.any(bq) or np.any(bk) or np.any(bv):
        return _numpy_fallback(query, mask, Wq, bq, Wk, bk, Wv, bv, Wo, bo)

    from concourse.bass_utils import run_bass_kernel_spmd

    nc = _get_nc()
    in_maps = make_in_maps(query, Wq, bq, Wk, bk, Wv, bv, Wo)
    res = run_bass_kernel_spmd(nc, in_maps, core_ids=list(range(NCORES)))
    return gather(res.results, bo)
